# revision 2
# baseline (speedup 1.0000x reference)
"""Trainium2 Bass kernel v2 for the MQA attention block (8 q-heads, shared K/V).

Sharding: 8 cores; core c -> batch b=c//4, query rows s0=(c%4)*512 .. +512,
all 8 heads.  K/V (full sequence, per batch) are computed redundantly on each
core; no cross-core communication.

v2 vs baseline:
- bf16 everywhere except PSUM accumulation, LN statistics and softmax
  tanh/exp intermediates (tolerance is 2e-2; bf16 keeps us ~3e-3).
- Batched LayerNorm/RoPE: stats and apply run on [P, 4, W] groups with
  broadcast APs instead of per-128-chunk scalar ops.
- Softmax drains PSUM early: DVE adds the attention bias (PSUM -> SBUF f32),
  tanh/exp run SBUF->SBUF on the scalar engine, exp writes bf16.
- Per-head q projection is software-pipelined INTO the attention loop so the
  PE keeps working while the scalar engine chews through tanh/exp.
- q-LN rsqrt during attention uses a DVE Newton iteration (no Act Sqrt) so
  the activation table never leaves exp/tanh mid-attention.
- attn@V moving dim 196 (192 v + 1 ones + 3 pad) instead of 256.
"""

import os
import sys

for _p in ("/opt/trn_rl_repo",):
    if _p not in sys.path and os.path.isdir(_p):
        sys.path.insert(0, _p)

import numpy as np
from contextlib import ExitStack

import concourse.bass as bass
import concourse.mybir as mybir
import concourse.tile as tile
from concourse import bacc
from concourse import bass_utils

F32 = mybir.dt.float32
BF16 = mybir.dt.bfloat16
U32 = mybir.dt.uint32

# problem shapes (hardcoded per contract)
B, S, D = 2, 2048, 1536
H, DQ, DK, DV = 8, 128, 128, 192
P = 128
SQ = S // 4          # 512 query rows per core
DC = D // P          # 12 contraction chunks
JC = S // P          # 16 key chunks
SC = SQ // P         # 4 query-row chunks
NCORES = 8
EPS_RMS = 1e-6
EPS_LN = 1e-5
SOFTCAP = 5.0
ROPE_BASE = 8192.0
HALF = DQ // 2       # 64
VPAD = 196           # v' row width: 192 v + 1 ones + 3 zero
JQ = 512             # keys per k/v projection quarter
NQ = S // JQ         # 4
LAG = 2              # attn@V emission lag (in 2-key-chunk groups)
JG = JC // 2         # 8 softmax groups per head
MAGIC = 0x5F3759DF   # rsqrt Newton seed

REPEAT = int(os.environ.get("KERNEL_REPEAT", "1"))


def build_program(repeat=None):
    global REPEAT
    if repeat is not None:
        REPEAT = repeat
    nc = bacc.Bacc(
        "TRN2", target_bir_lowering=False, debug=False, num_devices=NCORES
    )

    def din(name, shape, dt=BF16):
        return nc.dram_tensor(name, list(shape), dt, kind="ExternalInput").ap()

    # per-core inputs
    xT = din("xT", (D, S))
    xTq = din("xTq", (D, SQ))
    biasT = din("biasT", (S, SQ))
    cosq_t = din("cosq", (SQ, HALF))
    sinq_t = din("sinq", (SQ, HALF))
    # shared (replicated) inputs
    cosk_t = din("cosk", (S, HALF))
    sink_t = din("sink", (S, HALF))
    wq = din("wq", (D, H * DQ))
    wk = din("wk", (D, DK))
    wv = din("wv", (D, DV))
    wo = din("wo", (H * DV, D))
    bq_b = din("bq", (P, H), F32)
    bk_b = din("bk", (P, 1), F32)
    bva_b = din("bva", (P, 1), F32)
    bvb_b = din("bvb", (P, 1), F32)
    qgr_t = din("qgr", (P, DQ))
    qbr_t = din("qbr", (P, DQ))
    kgr_t = din("kgr", (P, DK))
    kbr_t = din("kbr", (P, DK))
    vgr_t = din("vgr", (P, DV))
    vbr_t = din("vbr", (P, DV))
    bor_t = din("bor", (P, D), F32)
    vpad_t = din("vpad", (P, JC * (VPAD - DV)))
    ident_t = din("identb", (P, P))
    out = nc.dram_tensor("out", [SQ, D], F32, kind="ExternalOutput").ap()

    TT = mybir.AluOpType
    AF = mybir.ActivationFunctionType
    AX = mybir.AxisListType

    with tile.TileContext(nc) as tc, ExitStack() as ctx:
        const = ctx.enter_context(tc.tile_pool(name="const", bufs=1))
        persist = ctx.enter_context(tc.tile_pool(name="persist", bufs=1))

        # ---- constants (live whole kernel); only the transpose identity is
        # fetched up front — everything else is emitted after the first
        # quarter's xt loads so the SP sequencer issues the critical-path
        # DMAs first (each DMACopy costs ~565ns of SP.SEQ issue time)
        ident_sb = const.tile([P, P], BF16)
        nc.sync.dma_start(ident_sb[:], ident_t)
        bk_sb = const.tile([P, 1], F32)
        nc.sync.dma_start(bk_sb[:], bk_b)
        bva_sb = const.tile([P, 1], F32)
        nc.sync.dma_start(bva_sb[:], bva_b)
        bvb_sb = const.tile([P, 1], F32)
        nc.sync.dma_start(bvb_sb[:], bvb_b)
        bq_sb = const.tile([P, H], F32)
        qgr = const.tile([P, DQ], BF16)
        qbr = const.tile([P, DQ], BF16)
        kgr = const.tile([P, DK], BF16)
        kbr = const.tile([P, DK], BF16)
        vgr = const.tile([P, DV], BF16)
        vbr = const.tile([P, DV], BF16)
        bor = const.tile([P, D], F32)
        cosk_sb = const.tile([P, JC, HALF], BF16)
        sink_sb = const.tile([P, JC, HALF], BF16)
        cosq_sb = const.tile([P, SC, HALF], BF16)
        sinq_sb = const.tile([P, SC, HALF], BF16)

        # persistent activation tensors
        kT_sb = persist.tile([P, S], BF16)            # rope'd LN'd k, [dk, s]
        vrow_sb = persist.tile([P, JC, VPAD], BF16)   # v rows + ones col
        qT_sb = persist.tile([P, H, SQ], BF16)        # rope'd LN'd q, [dq,h,i]
        yatt_sb = persist.tile([P, SC, H * DV], BF16)  # attn out rows
        biasT_sb = persist.tile([P, JC, SQ], BF16)
        xq_sb = persist.tile([P, DC, SQ], BF16)       # x columns for q proj
        wo_sb = persist.tile([P, DC, D], BF16)

        def load_consts():
            nc.sync.dma_start(
                cosk_sb[:], cosk_t.rearrange("(jc p) f -> p jc f", p=P)
            )
            nc.sync.dma_start(
                sink_sb[:], sink_t.rearrange("(jc p) f -> p jc f", p=P)
            )
            nc.sync.dma_start(kgr[:], kgr_t)
            nc.sync.dma_start(kbr[:], kbr_t)
            nc.sync.dma_start(vgr[:], vgr_t)
            nc.sync.dma_start(vbr[:], vbr_t)
            nc.sync.dma_start(
                vrow_sb[:, :, DV:],
                vpad_t.rearrange("p (jc f) -> p jc f", jc=JC),
            )
            nc.sync.dma_start(bq_sb[:], bq_b)
            nc.sync.dma_start(qgr[:], qgr_t)
            nc.sync.dma_start(qbr[:], qbr_t)
            nc.sync.dma_start(
                cosq_sb[:], cosq_t.rearrange("(sc p) f -> p sc f", p=P)
            )
            nc.sync.dma_start(
                sinq_sb[:], sinq_t.rearrange("(sc p) f -> p sc f", p=P)
            )
            nc.sync.dma_start(bor[:], bor_t)

        def newton_rsqrt(pool, e_tt, var_ap, G, tag):
            """y = var^-0.5 without Act: fixed-seed Newton iteration
            (y <- y*(1.5 - 0.5*v*y^2)).  Converges to <1e-4 relative in 4
            steps for v in [0.15, 1.3]; q-row variances here are ~0.4-0.9.
            tensor_scalar steps run on DVE, tensor_tensor steps on e_tt."""
            ve = nc.vector
            y = pool.tile([P, G], F32, tag=tag + "ny")
            ve.memset(y[:], 1.25)
            t = pool.tile([P, G], F32, tag=tag + "nt")
            for _ in range(4):
                e_tt.tensor_tensor(t[:], y[:], y[:], TT.mult)
                e_tt.tensor_tensor(t[:], t[:], var_ap, TT.mult)
                ve.tensor_scalar(t[:], t[:], -0.5, 1.5, TT.mult, TT.add)
                e_tt.tensor_tensor(y[:], y[:], t[:], TT.mult)
            return y

        def ln_batched(pool, x, G, W, grep, brep, rope, out_ap, tag, newton,
                       e_sq, e_small, e_apply, e_affine, e_rope):
            """LayerNorm each of the G rows-of-W in x ([P, G, W] bf16 SBUF),
            apply affine (grep/brep [P, W] row-replicated), optional rope
            ((cos, sin) [P, G, HALF] APs, W must be 128), write bf16 out_ap.
            The e_* args pick the engine (nc.vector / nc.gpsimd) per stage;
            the two reductions always run on DVE."""
            ve = nc.vector
            sq = pool.tile([P, G, W], BF16, tag=tag + "sq")
            e_sq.tensor_tensor(sq[:], x, x, TT.mult)
            s0 = pool.tile([P, G], F32, tag=tag + "s0")
            ve.tensor_reduce(s0[:], x, AX.X, TT.add)
            s1 = pool.tile([P, G], F32, tag=tag + "s1")
            ve.tensor_reduce(s1[:], sq[:], AX.X, TT.add)
            mu = pool.tile([P, G], F32, tag=tag + "mu")
            nc.vector.tensor_scalar(mu[:], s0[:], 1.0 / W, None, TT.mult)
            var = pool.tile([P, G], F32, tag=tag + "var")
            e_small.tensor_tensor(var[:], mu[:], mu[:], TT.mult)
            # var = (s1/W - mu^2) + eps
            nc.vector.scalar_tensor_tensor(
                var[:], s1[:], 1.0 / W, var[:], TT.mult, TT.subtract
            )
            nc.vector.tensor_scalar(var[:], var[:], EPS_LN, None, TT.add)
            if newton:
                rstd = newton_rsqrt(pool, e_small, var[:], G, tag)
            else:
                rstd = pool.tile([P, G], F32, tag=tag + "rstd")
                nc.scalar.activation(rstd[:], var[:], AF.Sqrt)
                ve.reciprocal(rstd[:], rstd[:])
            # center+scale per chunk with per-partition scalars (keeps the
            # big elementwise ops in the fast packed-bf16 DVE/Pool modes)
            xn = pool.tile([P, G, W], BF16, tag=tag + "xn")
            for g in range(G):
                nc.vector.tensor_scalar(
                    xn[:, g, :], x[:, g, :],
                    mu[:, g : g + 1], rstd[:, g : g + 1],
                    TT.subtract, TT.mult,
                )
            e_affine.tensor_tensor(
                xn[:], xn[:], grep.unsqueeze(1).broadcast_to([P, G, W]), TT.mult
            )
            brep_b = brep.unsqueeze(1).broadcast_to([P, G, W])
            if rope is None:
                e_affine.tensor_tensor(out_ap, xn[:], brep_b, TT.add)
                return
            e_affine.tensor_tensor(xn[:], xn[:], brep_b, TT.add)
            cos_ap, sin_ap = rope
            x1 = xn[:, :, :HALF]
            x2 = xn[:, :, HALF:]
            o1 = out_ap[:, :, :HALF]
            o2 = out_ap[:, :, HALF:]
            t1 = pool.tile([P, G, HALF], BF16, tag=tag + "t1")
            e_rope.tensor_tensor(o1, x1, cos_ap, TT.mult)
            e_rope.tensor_tensor(t1[:], x2, sin_ap, TT.mult)
            e_rope.tensor_tensor(o1, o1, t1[:], TT.subtract)
            e_rope.tensor_tensor(o2, x1, sin_ap, TT.mult)
            e_rope.tensor_tensor(t1[:], x2, cos_ap, TT.mult)
            e_rope.tensor_tensor(o2, o2, t1[:], TT.add)

        for _rep in range(REPEAT):
          with (
            tc.tile_pool(name="qpsp", bufs=1, space="PSUM") as qpsp,
            tc.tile_pool(name="ascr", bufs=1, space="PSUM") as ascr,
            tc.tile_pool(name="qw", bufs=2) as qw,
            tc.tile_pool(name="qst", bufs=2) as qst,
            tc.tile_pool(name="kvr", bufs=2) as kvr,
            tc.tile_pool(name="kvst", bufs=2) as kvst,
          ):
            # ---- per-head q-projection pipeline pieces (used during both
            # the KV phase (head 0) and the attention loop (heads 1..7))
            wq3 = wq.rearrange("(c p) f -> p c f", p=P)
            wq_tiles = {}
            qraw_tiles = {}
            qrow_tiles = {}

            def q_fetch(h):
                wqh = qw.tile([P, DC, DQ], BF16, tag="wqh")
                nc.sync.dma_start(
                    wqh[:], wq3[:, :, h * DQ : (h + 1) * DQ]
                )
                wq_tiles[h] = wqh

            def q_mm(h):
                wqh = wq_tiles.pop(h)
                q_ps = qpsp.tile([P, SQ], F32, tag="qps")
                for dc in range(DC):
                    nc.tensor.matmul(
                        q_ps[:], wqh[:, dc, :], xq_sb[:, dc, :],
                        start=(dc == 0), stop=(dc == DC - 1),
                    )
                qraw = qst.tile([P, SQ], BF16, tag="qraw")
                nc.vector.tensor_scalar_add(
                    qraw[:], q_ps[:], bq_sb[:, h : h + 1]
                )
                qraw_tiles[h] = qraw

            def q_rows(h):
                qraw = qraw_tiles.pop(h)
                scrq = ascr.tile([P, 512], BF16, tag="ascr", name="scrq")
                for t in range(4):
                    nc.tensor.transpose(
                        scrq[:, t * P : (t + 1) * P],
                        qraw[:, t * P : (t + 1) * P],
                        ident_sb[:],
                    )
                qrows = qst.tile([P, 4, P], BF16, tag="qrows")
                nc.vector.tensor_copy(
                    qrows[:], scrq[:].rearrange("p (t u) -> p t u", t=4)
                )
                qrow_tiles[h] = qrows

            def q_ln(h, on_pool=True):
                # heads 1..7 run during attention where DVE is busy -> Pool;
                # head 0 runs at the KV tail where Pool is busy -> DVE
                e = nc.gpsimd if on_pool else nc.vector
                qrows = qrow_tiles.pop(h)
                qrn = qst.tile([P, 4, P], BF16, tag="qrn")
                ln_batched(
                    qst, qrows[:], 4, DQ, qgr[:], qbr[:],
                    (cosq_sb[:, :, :], sinq_sb[:, :, :]),
                    qrn[:], "q", newton=True,
                    e_sq=e, e_small=e, e_apply=e, e_affine=e, e_rope=e,
                )
                scrq2 = ascr.tile([P, 512], BF16, tag="ascr", name="scrq2")
                for t in range(4):
                    nc.tensor.transpose(
                        scrq2[:, t * P : (t + 1) * P],
                        qrn[:, t, :],
                        ident_sb[:],
                    )
                nc.vector.tensor_copy(qT_sb[:, h, :], scrq2[:])

            # =====================================================
            # Phase KV: k/v projections + LN/rope, in S/4 quarters
            # =====================================================
            with (
                tc.tile_pool(name="kvw", bufs=1) as kvw,
                tc.tile_pool(name="kvx", bufs=3) as kvx,
                tc.tile_pool(name="kvps", bufs=1, space="PSUM") as kvps,
                tc.tile_pool(name="kvscr", bufs=2, space="PSUM") as kvscr,
            ):
                wk_sb = kvw.tile([P, DC, DK], BF16)
                nc.sync.dma_start(
                    wk_sb[:], wk.rearrange("(c p) f -> p c f", p=P)
                )
                wv_sb = kvw.tile([P, DC, DV], BF16)
                nc.sync.dma_start(
                    wv_sb[:], wv.rearrange("(c p) f -> p c f", p=P)
                )

                raws = {}

                def kv_mms(quar):
                    j0 = quar * JQ
                    kps = kvps.tile([P, JQ], F32, tag="kps")
                    vaps = kvps.tile([P, JQ], F32, tag="vaps")
                    vbps = kvps.tile([DV - P, JQ], F32, tag="vbps")
                    for dc in range(DC):
                        xt = kvx.tile([P, JQ], BF16, tag="xt")
                        nc.sync.dma_start(
                            xt[:], xT[dc * P : (dc + 1) * P, j0 : j0 + JQ]
                        )
                        nc.tensor.matmul(
                            kps[:], wk_sb[:, dc, :], xt[:],
                            start=(dc == 0), stop=(dc == DC - 1),
                        )
                        nc.tensor.matmul(
                            vaps[:], wv_sb[:, dc, :P], xt[:],
                            start=(dc == 0), stop=(dc == DC - 1),
                        )
                        nc.tensor.matmul(
                            vbps[:], wv_sb[:, dc, P:], xt[:],
                            start=(dc == 0), stop=(dc == DC - 1),
                        )
                    # drain+bias-fold on the (otherwise idle) scalar engine
                    kraw = kvr.tile([P, JQ], BF16, tag="kraw")
                    nc.scalar.activation(
                        kraw[:], kps[:], AF.Identity, bias=bk_sb[:, 0:1]
                    )
                    varaw = kvr.tile([P, JQ], BF16, tag="varaw")
                    nc.scalar.activation(
                        varaw[:], vaps[:], AF.Identity, bias=bva_sb[:, 0:1]
                    )
                    vbraw = kvr.tile([DV - P, JQ], BF16, tag="vbraw")
                    nc.scalar.activation(
                        vbraw[:], vbps[:], AF.Identity,
                        bias=bvb_sb[: DV - P, 0:1],
                    )
                    raws[quar] = (kraw, varaw, vbraw)

                def kv_rows(quar):
                    kraw, varaw, vbraw = raws.pop(quar)
                    j0 = quar * JQ
                    # ---- k rows: transpose 4 chunks, LN+rope, transpose back
                    scrk = kvscr.tile([P, 4 * DV], BF16, tag="scr", name="scrk")
                    for t in range(4):
                        nc.tensor.transpose(
                            scrk[:, t * P : (t + 1) * P],
                            kraw[:, t * P : (t + 1) * P],
                            ident_sb[:],
                        )
                    krows = kvr.tile([P, 4, P], BF16, tag="krows")
                    nc.vector.tensor_copy(
                        krows[:],
                        scrk[:, :JQ].rearrange("p (t u) -> p t u", t=4),
                    )
                    krn = kvr.tile([P, 4, P], BF16, tag="krn")
                    jc0 = quar * 4
                    ln_batched(
                        kvst, krows[:], 4, DK, kgr[:], kbr[:],
                        (
                            cosk_sb[:, jc0 : jc0 + 4, :],
                            sink_sb[:, jc0 : jc0 + 4, :],
                        ),
                        krn[:], "k", newton=False,
                        e_sq=nc.vector, e_small=nc.vector,
                        e_apply=nc.vector, e_affine=nc.vector,
                        e_rope=nc.vector,
                    )
                    # ---- v rows: transpose, LN (no rope) straight into vrow
                    scrv = kvscr.tile([P, 4 * DV], BF16, tag="scr", name="scrv")
                    for t in range(4):
                        nc.tensor.transpose(
                            scrv[:, t * DV : t * DV + P],
                            varaw[:, t * P : (t + 1) * P],
                            ident_sb[:],
                        )
                        nc.tensor.transpose(
                            scrv[:, t * DV + P : (t + 1) * DV],
                            vbraw[:, t * P : (t + 1) * P],
                            ident_sb[: DV - P, : DV - P],
                        )
                    vrows = kvr.tile([P, 4, DV], BF16, tag="vrows")
                    nc.vector.tensor_copy(
                        vrows[:], scrv[:].rearrange("p (t u) -> p t u", t=4)
                    )
                    ln_batched(
                        kvst, vrows[:], 4, DV, vgr[:], vbr[:], None,
                        vrow_sb[:, jc0 : jc0 + 4, :DV], "v", newton=False,
                        e_sq=nc.vector, e_small=nc.vector,
                        e_apply=nc.vector, e_affine=nc.vector,
                        e_rope=nc.vector,
                    )
                    # k transpose-back emitted after the next quarter's
                    # matmuls started; LN is long done by the time PE gets
                    # here, so no PE bubble
                    scrk2 = kvscr.tile(
                        [P, 4 * DV], BF16, tag="scr", name="scrk2"
                    )
                    for t in range(4):
                        nc.tensor.transpose(
                            scrk2[:, t * P : (t + 1) * P],
                            krn[:, t, :],
                            ident_sb[:],
                        )
                    nc.vector.tensor_copy(
                        kT_sb[:, j0 : j0 + JQ], scrk2[:, :JQ]
                    )

                for quar in range(NQ):
                    kv_mms(quar)
                    if quar == 0:
                        load_consts()
                        q_fetch(0)
                        nc.sync.dma_start(
                            xq_sb[:],
                            xTq.rearrange("(c p) s -> p c s", p=P),
                        )
                    if quar == 1:
                        nc.sync.dma_start(
                            biasT_sb[:],
                            biasT.rearrange("(jc p) i -> p jc i", p=P),
                        )
                    if quar == 2:
                        q_mm(0)
                    if quar == 3:
                        q_rows(0)
                    if quar > 0:
                        kv_rows(quar - 1)
                q_ln(0, on_pool=False)
                kv_rows(NQ - 1)
                q_fetch(1)

            # =====================================================
            # Attention with per-head q projection pipelined in
            # =====================================================
            with (
                tc.tile_pool(name="aps", bufs=1, space="PSUM") as aps,
                tc.tile_pool(name="yps", bufs=1, space="PSUM") as yps,
                tc.tile_pool(name="az", bufs=3) as az,
                tc.tile_pool(name="apt", bufs=1) as apt,
                tc.tile_pool(name="adr", bufs=2) as adr,
            ):
                # each y accumulator gets its own 2KB PSUM bank: start=True
                # zeroes a whole 2KB "zero region", so concurrently
                # accumulating tiles must never share one
                y_ps = [
                    yps.tile([P, 512], F32, name=f"y{ic}") for ic in range(SC)
                ]

                def attnv_half(pt, jp, half):
                    for c in range(half * 2, half * 2 + 2):
                        jc = jp * 4 + c
                        for ic in range(SC):
                            nc.tensor.matmul(
                                y_ps[ic][:, :VPAD],
                                pt[:, c, ic * P : (ic + 1) * P],
                                vrow_sb[:, jc, :],
                                start=(jc == 0),
                                stop=(jc == JC - 1),
                            )

                def drain_head(h):
                    # normalize rows by the ones-column sum
                    for ic in range(SC):
                        rec = adr.tile([P, 1], F32, tag="rec")
                        nc.vector.reciprocal(
                            rec[:], y_ps[ic][:, DV : DV + 1]
                        )
                        nc.vector.tensor_scalar(
                            yatt_sb[:, ic, h * DV : (h + 1) * DV],
                            y_ps[ic][:, :DV],
                            rec[:, 0:1], None, TT.mult,
                        )

                NJP = JC // 4  # 4 softmax emission units (4 key chunks each)
                for h in range(H):
                    pts = {}
                    for jp in range(NJP):
                        z = az.tile([P, 4, 512], F32, tag="z")
                        for half in range(2):
                            pq = aps.tile(
                                [P, 2, 512], F32, tag="pq", name="pq"
                            )
                            for c in range(2):
                                jc = jp * 4 + half * 2 + c
                                nc.tensor.matmul(
                                    pq[:, c, :],
                                    kT_sb[:, jc * P : (jc + 1) * P],
                                    qT_sb[:, h, :],
                                    start=True, stop=True,
                                )
                            jc0 = jp * 4 + half * 2
                            nc.vector.tensor_tensor(
                                z[:, half * 2 : half * 2 + 2, :], pq[:],
                                biasT_sb[:, jc0 : jc0 + 2, :],
                                TT.add,
                            )
                            # fill the pq WAR window with half of the lagged
                            # attn@V matmuls
                            if jp >= LAG:
                                attnv_half(pts[jp - LAG], jp - LAG, half)
                        if jp == 0 and h > 0:
                            # previous head's drain, emitted after this
                            # head's first bias-add so the scalar engine is
                            # never left waiting on the DVE
                            drain_head(h - 1)
                        if h == 0 and jp == 0:
                            # gate the big wo load behind attention start so
                            # it never competes with the KV-phase xt loads
                            nc.vector.memset(wo_sb[:, 0, 0:1], 0.0)
                            nc.sync.dma_start(
                                wo_sb[:],
                                wo.rearrange("(c p) f -> p c f", p=P),
                            )

                        nc.scalar.activation(
                            z[:], z[:], AF.Tanh, scale=1.0 / SOFTCAP
                        )
                        pt = apt.tile(
                            [P, 4, 512], BF16, tag=f"pt{jp % 3}",
                            name=f"pt{jp % 3}",
                        )
                        nc.scalar.activation(pt[:], z[:], AF.Exp, scale=SOFTCAP)
                        pts[jp] = pt
                        if h + 1 < H:
                            if jp == 1:
                                q_mm(h + 1)
                            elif jp == 2:
                                q_rows(h + 1)
                            elif jp == 3:
                                q_ln(h + 1)
                                if h + 2 < H:
                                    q_fetch(h + 2)
                        if jp >= LAG:
                            pts.pop(jp - LAG)
                    for jpr in range(NJP - LAG, NJP):
                        pt_t = pts.pop(jpr)
                        attnv_half(pt_t, jpr, 0)
                        attnv_half(pt_t, jpr, 1)
                drain_head(H - 1)

            # =====================================================
            # Output projection
            # =====================================================
            with (
                tc.tile_pool(name="oyT", bufs=1) as oyT,
                tc.tile_pool(name="op", bufs=2) as op,
                tc.tile_pool(name="ops", bufs=1, space="PSUM") as ops,
                tc.tile_pool(name="oscr", bufs=2, space="PSUM") as oscr,
            ):
                yT_sb = oyT.tile([P, DC, SQ], BF16)
                for sc in range(SC):
                    for fg in range(3):
                        scro = oscr.tile([P, 512], BF16, tag="oscr")
                        for t in range(4):
                            fc = fg * 4 + t
                            nc.tensor.transpose(
                                scro[:, t * P : (t + 1) * P],
                                yatt_sb[:, sc, fc * P : (fc + 1) * P],
                                ident_sb[:],
                            )
                        nc.vector.tensor_copy(
                            yT_sb[:, fg * 4 : (fg + 1) * 4,
                                  sc * P : (sc + 1) * P],
                            scro[:].rearrange("p (t u) -> p t u", t=4),
                        )
                for sc in range(SC):
                    o_ps = ops.tile([P, D], F32, tag="ops")
                    for fc in range(DC):
                        for n in range(D // 512):
                            nc.tensor.matmul(
                                o_ps[:, n * 512 : (n + 1) * 512],
                                yT_sb[:, fc, sc * P : (sc + 1) * P],
                                wo_sb[:, fc, n * 512 : (n + 1) * 512],
                                start=(fc == 0),
                                stop=(fc == DC - 1),
                            )
                    o_sb = op.tile([P, D], F32, tag="osb")
                    nc.vector.tensor_tensor(o_sb[:], o_ps[:], bor[:], TT.add)
                    nc.sync.dma_start(out[sc * P : (sc + 1) * P, :], o_sb[:])

    nc.compile()
    return nc


def _host_prep(inputs):
    import ml_dtypes

    bf = ml_dtypes.bfloat16
    f32 = np.float32
    x = np.asarray(inputs["x"], f32)
    bias = np.asarray(inputs["attention_bias"], f32)
    g1 = np.asarray(inputs["g1"], f32)
    b1 = np.asarray(inputs["b1"], f32)
    rr1 = np.asarray(inputs["rrms1"], f32)
    Wq = np.asarray(inputs["Wq"], f32)
    Wk = np.asarray(inputs["Wk"], f32)
    Wv = np.asarray(inputs["Wv"], f32)
    qg = np.asarray(inputs["qg"], f32)
    qb = np.asarray(inputs["qb"], f32)
    kg = np.asarray(inputs["kg"], f32)
    kb = np.asarray(inputs["kb"], f32)
    vg = np.asarray(inputs["vg"], f32)
    vb = np.asarray(inputs["vb"], f32)
    Wo = np.asarray(inputs["Wo"], f32)
    bo = np.asarray(inputs["bo"], f32)
    g2 = np.asarray(inputs["g2"], f32)
    b2 = np.asarray(inputs["b2"], f32)
    rr2 = np.asarray(inputs["rrms2"], f32)

    scale1 = (g1 * (1.0 / np.sqrt(rr1 + EPS_RMS))).astype(f32)
    Wq_e = Wq * scale1[:, None]
    Wk_e = Wk * scale1[:, None]
    Wv_e = Wv * scale1[:, None]
    bq_row = (b1 @ Wq).astype(f32)      # [H*DQ]
    bk_row = (b1 @ Wk).astype(f32)      # [DK]
    bv_row = (b1 @ Wv).astype(f32)      # [DV]
    sc_q = f32(DQ) ** f32(-0.5)
    qg_e = (qg * sc_q).astype(f32)
    qb_e = (qb * sc_q).astype(f32)
    scale2 = (g2 * (1.0 / np.sqrt(rr2 + EPS_RMS))).astype(f32)
    Wo_e = Wo * scale2[None, :]
    bo_e = (bo * scale2 + b2).astype(f32)

    freqs = (
        1.0 / (ROPE_BASE ** (np.arange(HALF, dtype=f32) / HALF))
    ).astype(f32)
    ang = np.arange(S, dtype=f32)[:, None] * freqs[None, :]
    cos = np.cos(ang).astype(f32)                        # [S, 64]
    sin = np.sin(ang).astype(f32)

    bva = bv_row[:P].reshape(P, 1)
    bvb = np.zeros((P, 1), f32)
    bvb[: DV - P, 0] = bv_row[P:]

    vpad = np.zeros((P, JC * (VPAD - DV)), f32)
    vpad[:, 0 :: (VPAD - DV)] = 1.0

    rep = lambda v: np.broadcast_to(v[None, :], (P, v.shape[0]))
    cbf = lambda a: np.ascontiguousarray(np.asarray(a, f32).astype(bf))
    cf = lambda a: np.ascontiguousarray(a, f32)
    shared = {
        "cosk": cbf(cos),
        "sink": cbf(sin),
        "wq": cbf(Wq_e),
        "wk": cbf(Wk_e),
        "wv": cbf(Wv_e),
        "wo": cbf(Wo_e),
        "bq": cf(bq_row.reshape(H, DQ).T),
        "bk": cf(bk_row.reshape(DK, 1)),
        "bva": cf(bva),
        "bvb": cf(bvb),
        "qgr": cbf(rep(qg_e)),
        "qbr": cbf(rep(qb_e)),
        "kgr": cbf(rep(kg)),
        "kbr": cbf(rep(kb)),
        "vgr": cbf(rep(vg)),
        "vbr": cbf(rep(vb)),
        "bor": cf(rep(bo_e)),
        "vpad": cbf(vpad),
        "identb": cbf(np.eye(P, dtype=f32)),
    }

    xTs = [np.ascontiguousarray(x[b].T).astype(bf) for b in range(B)]
    in_maps = []
    for c in range(NCORES):
        b = c // 4
        s0 = (c % 4) * SQ
        m = dict(shared)
        m["xT"] = np.ascontiguousarray(xTs[b])
        m["xTq"] = np.ascontiguousarray(xTs[b][:, s0 : s0 + SQ])
        m["biasT"] = cbf(bias[0, 0, s0 : s0 + SQ, :].T)
        m["cosq"] = cbf(cos[s0 : s0 + SQ, :])
        m["sinq"] = cbf(sin[s0 : s0 + SQ, :])
        in_maps.append(m)
    return in_maps


_NC_CACHE = None


def _get_nc():
    global _NC_CACHE
    if _NC_CACHE is None:
        _NC_CACHE = build_program()
    return _NC_CACHE


def kernel(**inputs) -> np.ndarray:
    nc = _get_nc()
    in_maps = _host_prep(inputs)
    res = bass_utils.run_bass_kernel_spmd(
        nc, in_maps, core_ids=list(range(NCORES))
    )
    outs = res.results
    full = np.empty((B, S, D), np.float32)
    for c in range(NCORES):
        b = c // 4
        s0 = (c % 4) * SQ
        full[b, s0 : s0 + SQ, :] = outs[c]["out"]
    return full


if __name__ == "__main__":
    nc = _get_nc()
    print("build + compile OK")


# revision 3
# speedup vs baseline: 1.0798x; 1.0798x over previous
"""Trainium2 Bass kernel v2 for the MQA attention block (8 q-heads, shared K/V).

Sharding: 8 cores; core c -> batch b=c//4, query rows s0=(c%4)*512 .. +512,
all 8 heads.  K/V (full sequence, per batch) are computed redundantly on each
core; no cross-core communication.

v2 vs baseline:
- bf16 everywhere except PSUM accumulation, LN statistics and softmax
  tanh/exp intermediates (tolerance is 2e-2; bf16 keeps us ~3e-3).
- Batched LayerNorm/RoPE: stats and apply run on [P, 4, W] groups with
  broadcast APs instead of per-128-chunk scalar ops.
- Softmax drains PSUM early: DVE adds the attention bias (PSUM -> SBUF f32),
  tanh/exp run SBUF->SBUF on the scalar engine, exp writes bf16.
- Per-head q projection is software-pipelined INTO the attention loop so the
  PE keeps working while the scalar engine chews through tanh/exp.
- q-LN rsqrt during attention uses a DVE Newton iteration (no Act Sqrt) so
  the activation table never leaves exp/tanh mid-attention.
- attn@V moving dim 196 (192 v + 1 ones + 3 pad) instead of 256.
"""

import os
import sys

for _p in ("/opt/trn_rl_repo",):
    if _p not in sys.path and os.path.isdir(_p):
        sys.path.insert(0, _p)

import numpy as np
from contextlib import ExitStack

import concourse.bass as bass
import concourse.mybir as mybir
import concourse.tile as tile
from concourse import bacc
from concourse import bass_utils

F32 = mybir.dt.float32
BF16 = mybir.dt.bfloat16
U32 = mybir.dt.uint32

# problem shapes (hardcoded per contract)
B, S, D = 2, 2048, 1536
H, DQ, DK, DV = 8, 128, 128, 192
P = 128
SQ = S // 4          # 512 query rows per core
DC = D // P          # 12 contraction chunks
JC = S // P          # 16 key chunks
SC = SQ // P         # 4 query-row chunks
NCORES = 8
EPS_RMS = 1e-6
EPS_LN = 1e-5
SOFTCAP = 5.0
ROPE_BASE = 8192.0
HALF = DQ // 2       # 64
VPAD = 196           # v' row width: 192 v + 1 ones + 3 zero
JQ = 512             # keys per k/v projection quarter
NQ = S // JQ         # 4
LAG = 2              # attn@V emission lag (in 2-key-chunk groups)
JG = JC // 2         # 8 softmax groups per head
MAGIC = 0x5F3759DF   # rsqrt Newton seed

REPEAT = int(os.environ.get("KERNEL_REPEAT", "1"))


def build_program(repeat=None):
    global REPEAT
    if repeat is not None:
        REPEAT = repeat
    nc = bacc.Bacc(
        "TRN2", target_bir_lowering=False, debug=False, num_devices=NCORES
    )

    def din(name, shape, dt=BF16):
        return nc.dram_tensor(name, list(shape), dt, kind="ExternalInput").ap()

    # per-core inputs
    xT = din("xT", (D, S))
    xTq = din("xTq", (D, SQ))
    biasT = din("biasT", (S, SQ))
    cosq_t = din("cosq", (SQ, HALF))
    sinq_t = din("sinq", (SQ, HALF))
    # shared (replicated) inputs
    cosk_t = din("cosk", (S, HALF))
    sink_t = din("sink", (S, HALF))
    wq = din("wq", (D, H * DQ))
    wk = din("wk", (D, DK))
    wv = din("wv", (D, DV))
    wo = din("wo", (H * DV, D))
    bq_b = din("bq", (P, H), F32)
    bk_b = din("bk", (P, 1), F32)
    bva_b = din("bva", (P, 1), F32)
    bvb_b = din("bvb", (P, 1), F32)
    qgr_t = din("qgr", (P, DQ))
    qbr_t = din("qbr", (P, DQ))
    kgr_t = din("kgr", (P, DK))
    kbr_t = din("kbr", (P, DK))
    vgr_t = din("vgr", (P, DV))
    vbr_t = din("vbr", (P, DV))
    bor_t = din("bor", (P, D), F32)
    vpad_t = din("vpad", (P, JC * (VPAD - DV)))
    ident_t = din("identb", (P, P))
    out = nc.dram_tensor("out", [SQ, D], F32, kind="ExternalOutput").ap()

    TT = mybir.AluOpType
    AF = mybir.ActivationFunctionType
    AX = mybir.AxisListType

    with tile.TileContext(nc) as tc, ExitStack() as ctx:
        const = ctx.enter_context(tc.tile_pool(name="const", bufs=1))
        persist = ctx.enter_context(tc.tile_pool(name="persist", bufs=1))

        # ---- constants (live whole kernel); only the transpose identity is
        # fetched up front — everything else is emitted after the first
        # quarter's xt loads so the SP sequencer issues the critical-path
        # DMAs first (each DMACopy costs ~565ns of SP.SEQ issue time)
        ident_sb = const.tile([P, P], BF16)
        nc.sync.dma_start(ident_sb[:], ident_t)
        bk_sb = const.tile([P, 1], F32)
        nc.sync.dma_start(bk_sb[:], bk_b)
        bva_sb = const.tile([P, 1], F32)
        nc.sync.dma_start(bva_sb[:], bva_b)
        bvb_sb = const.tile([P, 1], F32)
        nc.sync.dma_start(bvb_sb[:], bvb_b)
        bq_sb = const.tile([P, H], F32)
        qgr = const.tile([P, DQ], BF16)
        qbr = const.tile([P, DQ], BF16)
        kgr = const.tile([P, DK], BF16)
        kbr = const.tile([P, DK], BF16)
        vgr = const.tile([P, DV], BF16)
        vbr = const.tile([P, DV], BF16)
        bor = const.tile([P, D], F32)
        cosk_sb = const.tile([P, JC, HALF], BF16)
        sink_sb = const.tile([P, JC, HALF], BF16)
        cosq_sb = const.tile([P, SC, HALF], BF16)
        sinq_sb = const.tile([P, SC, HALF], BF16)

        # persistent activation tensors
        kT_sb = persist.tile([P, S], BF16)            # rope'd LN'd k, [dk, s]
        vrow_sb = persist.tile([P, JC, VPAD], BF16)   # v rows + ones col
        qT_sb = persist.tile([P, H, SQ], BF16)        # rope'd LN'd q, [dq,h,i]
        yatt_sb = persist.tile([P, SC, H * DV], BF16)  # attn out rows
        biasT_sb = persist.tile([P, JC, SQ], BF16)
        xq_sb = persist.tile([P, DC, SQ], BF16)       # x columns for q proj
        wo_sb = persist.tile([P, DC, D], BF16)
        yT0_sb = persist.tile([P, DC, SQ], BF16)      # attn out, transposed

        def load_consts():
            nc.sync.dma_start(
                cosk_sb[:], cosk_t.rearrange("(jc p) f -> p jc f", p=P)
            )
            nc.sync.dma_start(
                sink_sb[:], sink_t.rearrange("(jc p) f -> p jc f", p=P)
            )
            nc.sync.dma_start(kgr[:], kgr_t)
            nc.sync.dma_start(kbr[:], kbr_t)
            nc.sync.dma_start(vgr[:], vgr_t)
            nc.sync.dma_start(vbr[:], vbr_t)
            nc.sync.dma_start(
                vrow_sb[:, :, DV:],
                vpad_t.rearrange("p (jc f) -> p jc f", jc=JC),
            )
            nc.sync.dma_start(bq_sb[:], bq_b)
            nc.sync.dma_start(qgr[:], qgr_t)
            nc.sync.dma_start(qbr[:], qbr_t)
            nc.sync.dma_start(
                cosq_sb[:], cosq_t.rearrange("(sc p) f -> p sc f", p=P)
            )
            nc.sync.dma_start(
                sinq_sb[:], sinq_t.rearrange("(sc p) f -> p sc f", p=P)
            )
            nc.sync.dma_start(bor[:], bor_t)

        def newton_rsqrt(pool, e_tt, var_ap, G, tag):
            """y = var^-0.5 without Act: fixed-seed Newton iteration
            (y <- y*(1.5 - 0.5*v*y^2)).  Converges to <1e-4 relative in 4
            steps for v in [0.15, 1.3]; q-row variances here are ~0.4-0.9.
            tensor_scalar steps run on DVE, tensor_tensor steps on e_tt."""
            ve = nc.vector
            y = pool.tile([P, G], F32, tag=tag + "ny")
            ve.memset(y[:], 1.25)
            t = pool.tile([P, G], F32, tag=tag + "nt")
            for _ in range(4):
                e_tt.tensor_tensor(t[:], y[:], y[:], TT.mult)
                e_tt.tensor_tensor(t[:], t[:], var_ap, TT.mult)
                ve.tensor_scalar(t[:], t[:], -0.5, 1.5, TT.mult, TT.add)
                e_tt.tensor_tensor(y[:], y[:], t[:], TT.mult)
            return y

        def ln_batched(pool, x, G, W, grep, brep, rope, out_ap, tag, newton,
                       e_sq, e_small, e_apply, e_affine, e_rope):
            """LayerNorm each of the G rows-of-W in x ([P, G, W] bf16 SBUF),
            apply affine (grep/brep [P, W] row-replicated), optional rope
            ((cos, sin) [P, G, HALF] APs, W must be 128), write bf16 out_ap.
            The e_* args pick the engine (nc.vector / nc.gpsimd) per stage;
            the two reductions always run on DVE."""
            ve = nc.vector
            sq = pool.tile([P, G, W], BF16, tag=tag + "sq")
            e_sq.tensor_tensor(sq[:], x, x, TT.mult)
            s0 = pool.tile([P, G], F32, tag=tag + "s0")
            ve.tensor_reduce(s0[:], x, AX.X, TT.add)
            s1 = pool.tile([P, G], F32, tag=tag + "s1")
            ve.tensor_reduce(s1[:], sq[:], AX.X, TT.add)
            mu = pool.tile([P, G], F32, tag=tag + "mu")
            nc.vector.tensor_scalar(mu[:], s0[:], 1.0 / W, None, TT.mult)
            var = pool.tile([P, G], F32, tag=tag + "var")
            e_small.tensor_tensor(var[:], mu[:], mu[:], TT.mult)
            # var = (s1/W - mu^2) + eps
            nc.vector.scalar_tensor_tensor(
                var[:], s1[:], 1.0 / W, var[:], TT.mult, TT.subtract
            )
            nc.vector.tensor_scalar(var[:], var[:], EPS_LN, None, TT.add)
            if newton:
                rstd = newton_rsqrt(pool, e_small, var[:], G, tag)
            else:
                rstd = pool.tile([P, G], F32, tag=tag + "rstd")
                nc.scalar.activation(rstd[:], var[:], AF.Sqrt)
                ve.reciprocal(rstd[:], rstd[:])
            # center+scale per chunk with per-partition scalars (keeps the
            # big elementwise ops in the fast packed-bf16 DVE/Pool modes)
            xn = pool.tile([P, G, W], BF16, tag=tag + "xn")
            for g in range(G):
                nc.vector.tensor_scalar(
                    xn[:, g, :], x[:, g, :],
                    mu[:, g : g + 1], rstd[:, g : g + 1],
                    TT.subtract, TT.mult,
                )
            e_affine.tensor_tensor(
                xn[:], xn[:], grep.unsqueeze(1).broadcast_to([P, G, W]), TT.mult
            )
            brep_b = brep.unsqueeze(1).broadcast_to([P, G, W])
            if rope is None:
                e_affine.tensor_tensor(out_ap, xn[:], brep_b, TT.add)
                return
            e_affine.tensor_tensor(xn[:], xn[:], brep_b, TT.add)
            cos_ap, sin_ap = rope
            x1 = xn[:, :, :HALF]
            x2 = xn[:, :, HALF:]
            o1 = out_ap[:, :, :HALF]
            o2 = out_ap[:, :, HALF:]
            t1 = pool.tile([P, G, HALF], BF16, tag=tag + "t1")
            e_rope.tensor_tensor(o1, x1, cos_ap, TT.mult)
            e_rope.tensor_tensor(t1[:], x2, sin_ap, TT.mult)
            e_rope.tensor_tensor(o1, o1, t1[:], TT.subtract)
            e_rope.tensor_tensor(o2, x1, sin_ap, TT.mult)
            e_rope.tensor_tensor(t1[:], x2, cos_ap, TT.mult)
            e_rope.tensor_tensor(o2, o2, t1[:], TT.add)

        for _rep in range(REPEAT):
          with (
            tc.tile_pool(name="qpsp", bufs=1, space="PSUM") as qpsp,
            tc.tile_pool(name="ascr", bufs=1, space="PSUM") as ascr,
            tc.tile_pool(name="qw", bufs=2) as qw,
            tc.tile_pool(name="qst", bufs=2) as qst,
            tc.tile_pool(name="kvr", bufs=2) as kvr,
            tc.tile_pool(name="kvst", bufs=2) as kvst,
          ):
            # ---- per-head q-projection pipeline pieces (used during both
            # the KV phase (head 0) and the attention loop (heads 1..7))
            wq3 = wq.rearrange("(c p) f -> p c f", p=P)
            wq_tiles = {}
            qraw_tiles = {}
            qrow_tiles = {}

            def q_fetch(h):
                wqh = qw.tile([P, DC, DQ], BF16, tag="wqh")
                nc.sync.dma_start(
                    wqh[:], wq3[:, :, h * DQ : (h + 1) * DQ]
                )
                wq_tiles[h] = wqh

            def q_mm(h):
                wqh = wq_tiles.pop(h)
                q_ps = qpsp.tile([P, SQ], F32, tag="qps")
                for dc in range(DC):
                    nc.tensor.matmul(
                        q_ps[:], wqh[:, dc, :], xq_sb[:, dc, :],
                        start=(dc == 0), stop=(dc == DC - 1),
                    )
                qraw = qst.tile([P, SQ], BF16, tag="qraw")
                nc.vector.tensor_scalar_add(
                    qraw[:], q_ps[:], bq_sb[:, h : h + 1]
                )
                qraw_tiles[h] = qraw

            def q_rows(h):
                qraw = qraw_tiles.pop(h)
                scrq = ascr.tile([P, 512], BF16, tag="ascr", name="scrq")
                for t in range(4):
                    nc.tensor.transpose(
                        scrq[:, t * P : (t + 1) * P],
                        qraw[:, t * P : (t + 1) * P],
                        ident_sb[:],
                    )
                qrows = qst.tile([P, 4, P], BF16, tag="qrows")
                nc.vector.tensor_copy(
                    qrows[:], scrq[:].rearrange("p (t u) -> p t u", t=4)
                )
                qrow_tiles[h] = qrows

            def q_ln(h, on_pool=True):
                # heads 1..7 run during attention where DVE is busy -> Pool;
                # head 0 runs at the KV tail where Pool is busy -> DVE
                e = nc.gpsimd if on_pool else nc.vector
                qrows = qrow_tiles.pop(h)
                qrn = qst.tile([P, 4, P], BF16, tag="qrn")
                ln_batched(
                    qst, qrows[:], 4, DQ, qgr[:], qbr[:],
                    (cosq_sb[:, :, :], sinq_sb[:, :, :]),
                    qrn[:], "q", newton=True,
                    e_sq=e, e_small=e, e_apply=e, e_affine=e, e_rope=e,
                )
                scrq2 = ascr.tile([P, 512], BF16, tag="ascr", name="scrq2")
                for t in range(4):
                    nc.tensor.transpose(
                        scrq2[:, t * P : (t + 1) * P],
                        qrn[:, t, :],
                        ident_sb[:],
                    )
                nc.vector.tensor_copy(qT_sb[:, h, :], scrq2[:])

            # =====================================================
            # Phase KV: k/v projections + LN/rope, in S/4 quarters
            # =====================================================
            with (
                tc.tile_pool(name="kvw", bufs=1) as kvw,
                tc.tile_pool(name="kvx", bufs=6) as kvx,
                tc.tile_pool(name="kvps", bufs=1, space="PSUM") as kvps,
                tc.tile_pool(name="kvscr", bufs=2, space="PSUM") as kvscr,
            ):
                wk_sb = kvw.tile([P, DC, DK], BF16)
                nc.sync.dma_start(
                    wk_sb[:], wk.rearrange("(c p) f -> p c f", p=P)
                )
                wv_sb = kvw.tile([P, DC, DV], BF16)
                nc.sync.dma_start(
                    wv_sb[:], wv.rearrange("(c p) f -> p c f", p=P)
                )

                raws = {}

                def kv_mms(quar):
                    j0 = quar * JQ
                    kps = kvps.tile([P, JQ], F32, tag="kps")
                    vaps = kvps.tile([P, JQ], F32, tag="vaps")
                    vbps = kvps.tile([DV - P, JQ], F32, tag="vbps")
                    for dc in range(DC):
                        xt = kvx.tile([P, JQ], BF16, tag="xt")
                        nc.sync.dma_start(
                            xt[:], xT[dc * P : (dc + 1) * P, j0 : j0 + JQ]
                        )
                        nc.tensor.matmul(
                            kps[:], wk_sb[:, dc, :], xt[:],
                            start=(dc == 0), stop=(dc == DC - 1),
                        )
                        nc.tensor.matmul(
                            vaps[:], wv_sb[:, dc, :P], xt[:],
                            start=(dc == 0), stop=(dc == DC - 1),
                        )
                        nc.tensor.matmul(
                            vbps[:], wv_sb[:, dc, P:], xt[:],
                            start=(dc == 0), stop=(dc == DC - 1),
                        )
                    # drain+bias-fold on the (otherwise idle) scalar engine
                    kraw = kvr.tile([P, JQ], BF16, tag="kraw")
                    nc.scalar.activation(
                        kraw[:], kps[:], AF.Identity, bias=bk_sb[:, 0:1]
                    )
                    varaw = kvr.tile([P, JQ], BF16, tag="varaw")
                    nc.scalar.activation(
                        varaw[:], vaps[:], AF.Identity, bias=bva_sb[:, 0:1]
                    )
                    vbraw = kvr.tile([DV - P, JQ], BF16, tag="vbraw")
                    nc.scalar.activation(
                        vbraw[:], vbps[:], AF.Identity,
                        bias=bvb_sb[: DV - P, 0:1],
                    )
                    raws[quar] = (kraw, varaw, vbraw)

                def kv_rows(quar):
                    kraw, varaw, vbraw = raws.pop(quar)
                    j0 = quar * JQ
                    # ---- k rows: transpose 4 chunks, LN+rope, transpose back
                    scrk = kvscr.tile([P, 4 * DV], BF16, tag="scr", name="scrk")
                    for t in range(4):
                        nc.tensor.transpose(
                            scrk[:, t * P : (t + 1) * P],
                            kraw[:, t * P : (t + 1) * P],
                            ident_sb[:],
                        )
                    krows = kvr.tile([P, 4, P], BF16, tag="krows")
                    nc.vector.tensor_copy(
                        krows[:],
                        scrk[:, :JQ].rearrange("p (t u) -> p t u", t=4),
                    )
                    krn = kvr.tile([P, 4, P], BF16, tag="krn")
                    jc0 = quar * 4
                    ln_batched(
                        kvst, krows[:], 4, DK, kgr[:], kbr[:],
                        (
                            cosk_sb[:, jc0 : jc0 + 4, :],
                            sink_sb[:, jc0 : jc0 + 4, :],
                        ),
                        krn[:], "k", newton=False,
                        e_sq=nc.vector, e_small=nc.vector,
                        e_apply=nc.vector, e_affine=nc.vector,
                        e_rope=nc.vector,
                    )
                    # ---- v rows: transpose, LN (no rope) straight into vrow
                    scrv = kvscr.tile([P, 4 * DV], BF16, tag="scr", name="scrv")
                    for t in range(4):
                        nc.tensor.transpose(
                            scrv[:, t * DV : t * DV + P],
                            varaw[:, t * P : (t + 1) * P],
                            ident_sb[:],
                        )
                        nc.tensor.transpose(
                            scrv[:, t * DV + P : (t + 1) * DV],
                            vbraw[:, t * P : (t + 1) * P],
                            ident_sb[: DV - P, : DV - P],
                        )
                    vrows = kvr.tile([P, 4, DV], BF16, tag="vrows")
                    nc.vector.tensor_copy(
                        vrows[:], scrv[:].rearrange("p (t u) -> p t u", t=4)
                    )
                    ln_batched(
                        kvst, vrows[:], 4, DV, vgr[:], vbr[:], None,
                        vrow_sb[:, jc0 : jc0 + 4, :DV], "v", newton=False,
                        e_sq=nc.vector, e_small=nc.vector,
                        e_apply=nc.vector, e_affine=nc.vector,
                        e_rope=nc.vector,
                    )
                    # k transpose-back emitted after the next quarter's
                    # matmuls started; LN is long done by the time PE gets
                    # here, so no PE bubble
                    scrk2 = kvscr.tile(
                        [P, 4 * DV], BF16, tag="scr", name="scrk2"
                    )
                    for t in range(4):
                        nc.tensor.transpose(
                            scrk2[:, t * P : (t + 1) * P],
                            krn[:, t, :],
                            ident_sb[:],
                        )
                    nc.vector.tensor_copy(
                        kT_sb[:, j0 : j0 + JQ], scrk2[:, :JQ]
                    )

                for quar in range(NQ):
                    kv_mms(quar)
                    if quar == 0:
                        q_fetch(0)
                        nc.sync.dma_start(
                            xq_sb[:],
                            xTq.rearrange("(c p) s -> p c s", p=P),
                        )
                        load_consts()
                    if quar == 1:
                        nc.sync.dma_start(
                            biasT_sb[:],
                            biasT.rearrange("(jc p) i -> p jc i", p=P),
                        )
                    if quar == 2:
                        q_mm(0)
                    if quar == 3:
                        q_rows(0)
                    if quar > 0:
                        kv_rows(quar - 1)
                q_ln(0, on_pool=False)
                kv_rows(NQ - 1)
                q_fetch(1)

            # =====================================================
            # Attention with per-head q projection pipelined in
            # =====================================================
            with (
                tc.tile_pool(name="aps", bufs=1, space="PSUM") as aps,
                tc.tile_pool(name="yps", bufs=1, space="PSUM") as yps,
                tc.tile_pool(name="az", bufs=3) as az,
                tc.tile_pool(name="apt", bufs=1) as apt,
                tc.tile_pool(name="adr", bufs=2) as adr,
            ):
                # each y accumulator gets its own 2KB PSUM bank: start=True
                # zeroes a whole 2KB "zero region", so concurrently
                # accumulating tiles must never share one
                y_ps = [
                    yps.tile([P, 512], F32, name=f"y{ic}") for ic in range(SC)
                ]

                def attnv_half(pt, jp, half):
                    for c in range(half * 2, half * 2 + 2):
                        jc = jp * 4 + c
                        for ic in range(SC):
                            nc.tensor.matmul(
                                y_ps[ic][:, :VPAD],
                                pt[:, c, ic * P : (ic + 1) * P],
                                vrow_sb[:, jc, :],
                                start=(jc == 0),
                                stop=(jc == JC - 1),
                            )

                def drain_head(h):
                    # normalize rows by the ones-column sum
                    for ic in range(SC):
                        rec = adr.tile([P, 1], F32, tag="rec")
                        nc.vector.reciprocal(
                            rec[:], y_ps[ic][:, DV : DV + 1]
                        )
                        nc.vector.tensor_scalar(
                            yatt_sb[:, ic, h * DV : (h + 1) * DV],
                            y_ps[ic][:, :DV],
                            rec[:, 0:1], None, TT.mult,
                        )

                NJP = JC // 4  # 4 softmax emission units (4 key chunks each)
                for h in range(H):
                    pts = {}
                    for jp in range(NJP):
                        z = az.tile([P, 4, 512], F32, tag="z")
                        for half in range(2):
                            pq = aps.tile(
                                [P, 2, 512], F32, tag="pq", name="pq"
                            )
                            for c in range(2):
                                jc = jp * 4 + half * 2 + c
                                nc.tensor.matmul(
                                    pq[:, c, :],
                                    kT_sb[:, jc * P : (jc + 1) * P],
                                    qT_sb[:, h, :],
                                    start=True, stop=True,
                                )
                            jc0 = jp * 4 + half * 2
                            nc.vector.tensor_tensor(
                                z[:, half * 2 : half * 2 + 2, :], pq[:],
                                biasT_sb[:, jc0 : jc0 + 2, :],
                                TT.add,
                            )
                            # fill the pq WAR window with half of the lagged
                            # attn@V matmuls
                            if jp >= LAG:
                                attnv_half(pts[jp - LAG], jp - LAG, half)
                        if jp == 0 and h > 0:
                            # previous head's drain, emitted after this
                            # head's first bias-add so the scalar engine is
                            # never left waiting on the DVE
                            drain_head(h - 1)
                        if h == 0 and jp == 0:
                            # gate the big wo load behind attention start so
                            # it never competes with the KV-phase xt loads
                            nc.vector.memset(wo_sb[:, 0, 0:1], 0.0)
                            nc.sync.dma_start(
                                wo_sb[:],
                                wo.rearrange("(c p) f -> p c f", p=P),
                            )

                        nc.scalar.activation(
                            z[:], z[:], AF.Tanh, scale=1.0 / SOFTCAP
                        )
                        pt = apt.tile(
                            [P, 4, 512], BF16, tag=f"pt{jp % 3}",
                            name=f"pt{jp % 3}",
                        )
                        nc.scalar.activation(pt[:], z[:], AF.Exp, scale=SOFTCAP)
                        pts[jp] = pt
                        if h + 1 < H:
                            if jp == 1:
                                q_mm(h + 1)
                            elif jp == 2:
                                q_rows(h + 1)
                            elif jp == 3:
                                q_ln(h + 1)
                                if h + 2 < H:
                                    q_fetch(h + 2)
                        if h == H - 1 and jp in (1, 2):
                            fg = jp - 1
                            for sc in range(SC):
                                scry = ascr.tile(
                                    [P, 512], BF16, tag="ascr",
                                    name=f"scry{fg}{sc}",
                                )
                                for t in range(4):
                                    fc = fg * 4 + t
                                    nc.tensor.transpose(
                                        scry[:, t * P : (t + 1) * P],
                                        yatt_sb[:, sc, fc * P : (fc + 1) * P],
                                        ident_sb[:],
                                    )
                                nc.vector.tensor_copy(
                                    yT0_sb[:, fg * 4 : (fg + 1) * 4,
                                           sc * P : (sc + 1) * P],
                                    scry[:].rearrange(
                                        "p (t u) -> p t u", t=4
                                    ),
                                )
                        if jp >= LAG:
                            pts.pop(jp - LAG)
                    for jpr in range(NJP - LAG, NJP):
                        pt_t = pts.pop(jpr)
                        attnv_half(pt_t, jpr, 0)
                        attnv_half(pt_t, jpr, 1)
                drain_head(H - 1)

            # =====================================================
            # Output projection
            # =====================================================
            with (
                tc.tile_pool(name="op", bufs=2) as op,
                tc.tile_pool(name="ops", bufs=1, space="PSUM") as ops,
                tc.tile_pool(name="oscr", bufs=2, space="PSUM") as oscr,
            ):
                yT_sb = yT0_sb
                for sc in range(SC):
                    for fg in (2,):
                        scro = oscr.tile([P, 512], BF16, tag="oscr")
                        for t in range(4):
                            fc = fg * 4 + t
                            nc.tensor.transpose(
                                scro[:, t * P : (t + 1) * P],
                                yatt_sb[:, sc, fc * P : (fc + 1) * P],
                                ident_sb[:],
                            )
                        nc.vector.tensor_copy(
                            yT_sb[:, fg * 4 : (fg + 1) * 4,
                                  sc * P : (sc + 1) * P],
                            scro[:].rearrange("p (t u) -> p t u", t=4),
                        )
                for sc in range(SC):
                    o_ps = ops.tile([P, D], F32, tag="ops")
                    for fc in range(DC):
                        for n in range(D // 512):
                            nc.tensor.matmul(
                                o_ps[:, n * 512 : (n + 1) * 512],
                                yT_sb[:, fc, sc * P : (sc + 1) * P],
                                wo_sb[:, fc, n * 512 : (n + 1) * 512],
                                start=(fc == 0),
                                stop=(fc == DC - 1),
                            )
                    o_sb = op.tile([P, D], F32, tag="osb")
                    nc.vector.tensor_tensor(o_sb[:], o_ps[:], bor[:], TT.add)
                    nc.sync.dma_start(out[sc * P : (sc + 1) * P, :], o_sb[:])

    nc.compile()
    return nc


def _host_prep(inputs):
    import ml_dtypes

    bf = ml_dtypes.bfloat16
    f32 = np.float32
    x = np.asarray(inputs["x"], f32)
    bias = np.asarray(inputs["attention_bias"], f32)
    g1 = np.asarray(inputs["g1"], f32)
    b1 = np.asarray(inputs["b1"], f32)
    rr1 = np.asarray(inputs["rrms1"], f32)
    Wq = np.asarray(inputs["Wq"], f32)
    Wk = np.asarray(inputs["Wk"], f32)
    Wv = np.asarray(inputs["Wv"], f32)
    qg = np.asarray(inputs["qg"], f32)
    qb = np.asarray(inputs["qb"], f32)
    kg = np.asarray(inputs["kg"], f32)
    kb = np.asarray(inputs["kb"], f32)
    vg = np.asarray(inputs["vg"], f32)
    vb = np.asarray(inputs["vb"], f32)
    Wo = np.asarray(inputs["Wo"], f32)
    bo = np.asarray(inputs["bo"], f32)
    g2 = np.asarray(inputs["g2"], f32)
    b2 = np.asarray(inputs["b2"], f32)
    rr2 = np.asarray(inputs["rrms2"], f32)

    scale1 = (g1 * (1.0 / np.sqrt(rr1 + EPS_RMS))).astype(f32)
    Wq_e = Wq * scale1[:, None]
    Wk_e = Wk * scale1[:, None]
    Wv_e = Wv * scale1[:, None]
    bq_row = (b1 @ Wq).astype(f32)      # [H*DQ]
    bk_row = (b1 @ Wk).astype(f32)      # [DK]
    bv_row = (b1 @ Wv).astype(f32)      # [DV]
    sc_q = f32(DQ) ** f32(-0.5)
    qg_e = (qg * sc_q).astype(f32)
    qb_e = (qb * sc_q).astype(f32)
    scale2 = (g2 * (1.0 / np.sqrt(rr2 + EPS_RMS))).astype(f32)
    Wo_e = Wo * scale2[None, :]
    bo_e = (bo * scale2 + b2).astype(f32)

    freqs = (
        1.0 / (ROPE_BASE ** (np.arange(HALF, dtype=f32) / HALF))
    ).astype(f32)
    ang = np.arange(S, dtype=f32)[:, None] * freqs[None, :]
    cos = np.cos(ang).astype(f32)                        # [S, 64]
    sin = np.sin(ang).astype(f32)

    bva = bv_row[:P].reshape(P, 1)
    bvb = np.zeros((P, 1), f32)
    bvb[: DV - P, 0] = bv_row[P:]

    vpad = np.zeros((P, JC * (VPAD - DV)), f32)
    vpad[:, 0 :: (VPAD - DV)] = 1.0

    rep = lambda v: np.broadcast_to(v[None, :], (P, v.shape[0]))
    cbf = lambda a: np.ascontiguousarray(np.asarray(a, f32).astype(bf))
    cf = lambda a: np.ascontiguousarray(a, f32)
    shared = {
        "cosk": cbf(cos),
        "sink": cbf(sin),
        "wq": cbf(Wq_e),
        "wk": cbf(Wk_e),
        "wv": cbf(Wv_e),
        "wo": cbf(Wo_e),
        "bq": cf(bq_row.reshape(H, DQ).T),
        "bk": cf(bk_row.reshape(DK, 1)),
        "bva": cf(bva),
        "bvb": cf(bvb),
        "qgr": cbf(rep(qg_e)),
        "qbr": cbf(rep(qb_e)),
        "kgr": cbf(rep(kg)),
        "kbr": cbf(rep(kb)),
        "vgr": cbf(rep(vg)),
        "vbr": cbf(rep(vb)),
        "bor": cf(rep(bo_e)),
        "vpad": cbf(vpad),
        "identb": cbf(np.eye(P, dtype=f32)),
    }

    xTs = [np.ascontiguousarray(x[b].T).astype(bf) for b in range(B)]
    in_maps = []
    for c in range(NCORES):
        b = c // 4
        s0 = (c % 4) * SQ
        m = dict(shared)
        m["xT"] = np.ascontiguousarray(xTs[b])
        m["xTq"] = np.ascontiguousarray(xTs[b][:, s0 : s0 + SQ])
        m["biasT"] = cbf(bias[0, 0, s0 : s0 + SQ, :].T)
        m["cosq"] = cbf(cos[s0 : s0 + SQ, :])
        m["sinq"] = cbf(sin[s0 : s0 + SQ, :])
        in_maps.append(m)
    return in_maps


_NC_CACHE = None


def _get_nc():
    global _NC_CACHE
    if _NC_CACHE is None:
        _NC_CACHE = build_program()
    return _NC_CACHE


def kernel(**inputs) -> np.ndarray:
    nc = _get_nc()
    in_maps = _host_prep(inputs)
    res = bass_utils.run_bass_kernel_spmd(
        nc, in_maps, core_ids=list(range(NCORES))
    )
    outs = res.results
    full = np.empty((B, S, D), np.float32)
    for c in range(NCORES):
        b = c // 4
        s0 = (c % 4) * SQ
        full[b, s0 : s0 + SQ, :] = outs[c]["out"]
    return full


if __name__ == "__main__":
    nc = _get_nc()
    print("build + compile OK")
